# revision 51
# baseline (speedup 1.0000x reference)
"""Gated Linear Attention forward on 8 Trainium2 NeuronCores (Bass/Tile).

Problem: B=4, T=1024, D=1024, H=8, DK=64, DV=128, conv4 on q/k/v, low-rank
log-sigmoid forget gate, recurrent scan, RMS-norm + swish output gate, out proj.

Sharding: core = 2*b + hg  (b = batch, hg = half of the heads).
Each core computes its batch's tokens for 4 heads end-to-end and a partial
output projection (Wo row-block); the host sums the two partials per batch.

v2 (bf16 rewrite of the f32r baseline):
- All matmul operands and most element-wise traffic are bf16 (PSUM stays f32),
  which removes the f32r small-free-dim 4x matmul penalty, halves LDWEIGHTS,
  doubles DVE throughput, and halves all DMA bytes.
- Every DRAM tensor is pre-arranged on the host into its exact SBUF layout so
  each DMA is 128 long contiguous descriptors (the baseline's 4KB-descriptor
  rearranges left the PE idle for the first 37us).
- silu runs as a single ACT Silu op straight out of PSUM; ACT ops are grouped
  by table set (natural_log_exp | silu | natural_log_exp) so the activation
  tables load ~3x instead of 15x.
- All v / k^ transposes are hoisted out of the recurrent chunk loop; the output
  projection for chunk c is emitted right after chunk c's recurrence so the PE
  stream stays dense (HAM stays un-throttled).
- conv runs on the PE as 4 shifted diagonal matmuls; the diagonal matrices are
  built on-device from a [128,8,4] tap table instead of DMAing 2MB of zeros.
"""

import numpy as np
import ml_dtypes

import concourse.bass as bass
import concourse.mybir as mybir
import concourse.tile as tile
from concourse import bacc
from concourse.bass_utils import run_bass_kernel_spmd

F32 = mybir.dt.float32
BF16 = mybir.dt.bfloat16
AF = mybir.ActivationFunctionType
OP = mybir.AluOpType

# problem constants (hardcoded per the task contract)
B, T, D, H = 4, 1024, 1024, 8
KD, VD = 512, 1024
DK, DV = 64, 128
CONV = 4
GATE_NORM = 16.0
EPS = 1e-5
LN8 = float(np.log(8.0))

# per-core shapes
KDC, VDC = 256, 512          # q/k and v/gate channels per core
MIQ, MIV = 2, 4              # 128-wide channel tiles for q/k and v
C, NCH = 128, 8              # chunk length, number of chunks
G = 2                        # head groups of 2 heads (128 chans) per core
NCORES = 8

BF = ml_dtypes.bfloat16


def build_program():
    nc = bacc.Bacc("TRN2", target_bir_lowering=False, debug=False)

    # ---- DRAM I/O (all host-pre-arranged to SBUF layout, bf16) -------------
    srcT_d = nc.dram_tensor("srcT_in", [128, 8, T], BF16, kind="ExternalInput")
    wq_d = nc.dram_tensor("wq", [128, 8, KDC], BF16, kind="ExternalInput")
    wk_d = nc.dram_tensor("wk", [128, 8, KDC], BF16, kind="ExternalInput")
    wv_d = nc.dram_tensor("wv", [128, 8, VDC], BF16, kind="ExternalInput")
    wgate_d = nc.dram_tensor("wgate", [128, 8, VDC], BF16, kind="ExternalInput")
    wg1_d = nc.dram_tensor("wg1", [128, 8, 16], BF16, kind="ExternalInput")
    wg2b_d = nc.dram_tensor("wg2b", [17, KDC], BF16, kind="ExternalInput")
    wo_d = nc.dram_tensor("wo", [128, MIV, D], BF16, kind="ExternalInput")
    convw_d = nc.dram_tensor("convw", [128, 2 * MIQ + MIV, CONV], F32,
                             kind="ExternalInput")
    maskc_d = nc.dram_tensor("maskc", [128, NCH], F32, kind="ExternalInput")
    out_d = nc.dram_tensor("out", [NCH, 128, D], BF16, kind="ExternalOutput")

    ident_np = np.eye(128, dtype=np.float32)
    u = np.triu(np.ones((128, 128), np.float32))  # U[s,t] = 1 iff s <= t
    ident_d = nc.inline_tensor(ident_np, "ident_c")
    triu2_d = nc.inline_tensor(np.concatenate([u, u], axis=1), "triu2_c")

    # ---- static SBUF -------------------------------------------------------
    srcT = nc.alloc_sbuf_tensor("srcT", [128, 8, T], BF16)       # src^T, d-major
    q_sb = nc.alloc_sbuf_tensor("q_sb", [128, MIQ, T], BF16)     # q then q~
    k_sb = nc.alloc_sbuf_tensor("k_sb", [128, MIQ, T], BF16)     # k then k~
    v_sb = nc.alloc_sbuf_tensor("v_sb", [128, MIV, T], BF16)     # chan-major v
    gate_sb = nc.alloc_sbuf_tensor("gate_sb", [128, NCH, VDC], BF16)
    vnat = nc.alloc_sbuf_tensor("vnat", [128, NCH, VDC], BF16)   # time-major v
    khnat = nc.alloc_sbuf_tensor("khnat", [128, NCH, KDC], BF16)  # time-major k^
    xgT = nc.alloc_sbuf_tensor("xgT", [17, T], BF16)             # (src@Wg1)^T+1s
    spT = nc.alloc_sbuf_tensor("spT", [128, MIQ, T], F32)        # softplus(-gk)
    bsum = nc.alloc_sbuf_tensor("bsum", [128, MIQ, T], F32)      # chunk cumsum
    bCn = nc.alloc_sbuf_tensor("bCn", [128, MIQ, NCH], F32)
    Eall = nc.alloc_sbuf_tensor("Eall", [128, MIQ, NCH], F32)    # exp(b_C)
    texp_all = nc.alloc_sbuf_tensor("texp_all", [128, 4, T], BF16)
    ssq_all = nc.alloc_sbuf_tensor("ssq_all", [128, NCH * 4], F32)
    rrms_all = nc.alloc_sbuf_tensor("rrms_all", [128, NCH * 4], F32)
    wo_sb = nc.alloc_sbuf_tensor("wo_sb", [128, MIV, D], BF16)
    wgate_sb = nc.alloc_sbuf_tensor("wgate_sb", [128, 8, VDC], BF16)
    wg1_sb = nc.alloc_sbuf_tensor("wg1_sb", [128, 8, 16], BF16)
    wg2b_sb = nc.alloc_sbuf_tensor("wg2b_sb", [17, KDC], BF16)
    convw_sb = nc.alloc_sbuf_tensor("convw_sb", [128, 2 * MIQ + MIV, CONV], F32)
    maskc_sb = nc.alloc_sbuf_tensor("maskc_sb", [128, NCH], F32)
    ident_f = nc.alloc_sbuf_tensor("ident_f", [128, 128], F32)
    triu2_f = nc.alloc_sbuf_tensor("triu2_f", [128, 256], F32)
    identb = nc.alloc_sbuf_tensor("identb", [128, 128], BF16)
    triu2b = nc.alloc_sbuf_tensor("triu2b", [128, 256], BF16)
    dg_all = nc.alloc_sbuf_tensor("dg_all", [128, (2 * MIQ + MIV) * CONV, 128],
                                  mybir.dt.float32r)
    ones_sb = nc.alloc_sbuf_tensor("ones_sb", [128, 128], F32)
    ogT = nc.alloc_sbuf_tensor("ogT", [128, MIV, T], BF16)
    Sblk = [nc.alloc_sbuf_tensor(f"Sblk{g}", [128, 256], BF16) for g in range(G)]
    qblk = [nc.alloc_sbuf_tensor(f"qblk{g}", [128, 256], BF16) for g in range(G)]
    negln8 = nc.alloc_sbuf_tensor("negln8", [128, 1], F32)
    eps_col = nc.alloc_sbuf_tensor("eps_col", [128, 1], F32)

    with tile.TileContext(nc) as tc:
        with (
            tc.tile_pool(name="scr", bufs=4) as scr,
        ):
            # ---- phase 0: constants + input DMAs (one straight transfer per
            # tensor; src split by partition quarters across four queues) ----
            nc.sync.dma_start(out=ident_f[:], in_=ident_d[:])
            nc.sync.dma_start(out=triu2_f[:], in_=triu2_d[:])
            nc.sync.dma_start(out=wg1_sb[:], in_=wg1_d[:])
            nc.scalar.dma_start(out=wg2b_sb[:], in_=wg2b_d[:])
            # src sliced by kt pairs so per-kt accumulation starts early
            nc.sync.dma_start(out=srcT[:, 0:2, :], in_=srcT_d[:, 0:2, :])
            nc.scalar.dma_start(out=srcT[:, 2:4, :], in_=srcT_d[:, 2:4, :])
            nc.gpsimd.dma_start(out=srcT[:, 4:6, :], in_=srcT_d[:, 4:6, :])
            nc.sync.dma_start(out=srcT[:, 6:8, :], in_=srcT_d[:, 6:8, :])
            nc.scalar.dma_start(out=convw_sb[:], in_=convw_d[:])
            nc.sync.dma_start(out=maskc_sb[:], in_=maskc_d[:])

            nc.vector.memset(ones_sb[:], 1.0)
            nc.vector.memset(negln8[:], -LN8)
            nc.vector.memset(eps_col[:], EPS)
            # row 16 is the bias ones-row; rows 0..15 are overwritten later
            nc.vector.memset(xgT[:], 1.0)
            for g in range(G):
                nc.vector.memset(Sblk[g][:], 0.0)
                nc.vector.memset(qblk[g][:], 0.0)
            nc.vector.tensor_copy(out=identb[:], in_=ident_f[:])
            nc.vector.tensor_copy(out=triu2b[:], in_=triu2_f[:])

            wq_sb = nc.alloc_sbuf_tensor("wq_sb", [128, 8, KDC], BF16)
            wk_sb = nc.alloc_sbuf_tensor("wk_sb", [128, 8, KDC], BF16)
            wv_sb = nc.alloc_sbuf_tensor("wv_sb", [128, 8, VDC], BF16)
            nc.sync.dma_start(out=wq_sb[:], in_=wq_d[:])
            nc.scalar.dma_start(out=wk_sb[:], in_=wk_d[:])
            nc.gpsimd.dma_start(out=wv_sb[:], in_=wv_d[:])
            nc.sync.dma_start(out=wgate_sb[:], in_=wgate_d[:])
            nc.gpsimd.dma_start(out=wo_sb[:], in_=wo_d[:])

            # conv diag matrices built on-device: dg[ti*4+j] = diag(w[:, ti, j])
            for ti in range(2 * MIQ + MIV):
                for j in range(CONV):
                    nc.vector.tensor_scalar_mul(
                        dg_all[:, ti * CONV + j, :], ident_f[:],
                        convw_sb[:, ti, j:j + 1],
                    )

            # ---- gk path + projections + conv + gate -----------------------
            with (
                tc.tile_pool(name="scr2", bufs=2) as scr2,
                tc.tile_pool(name="ps_proj", bufs=6, space="PSUM") as ps_proj,
                tc.tile_pool(name="ps_tr", bufs=2, space="PSUM") as ps_tr,
            ):
                # kt-wave: xg and the q projection accumulate together so
                # each arriving src kt-slice feeds 6 back-to-back matmuls
                pxg = [ps_proj.tile([128, 512], F32, name="pp_xg", tag="pp")
                       for _ in range(2)]
                pq = [ps_proj.tile([128, 512], F32, name="pp_q", tag="pp")
                      for _ in range(4)]
                for kt in range(8):
                    for nh in range(2):
                        nc.tensor.matmul(
                            pxg[nh][0:16, :],
                            wg1_sb[:, kt, :],
                            srcT[:, kt, nh * 512:(nh + 1) * 512],
                            start=(kt == 0),
                            stop=(kt == 7),
                        )
                    for mi in range(MIQ):
                        for nh in range(2):
                            nc.tensor.matmul(
                                pq[mi * 2 + nh][:],
                                wq_sb[:, kt, mi * 128:(mi + 1) * 128],
                                srcT[:, kt, nh * 512:(nh + 1) * 512],
                                start=(kt == 0),
                                stop=(kt == 7),
                            )
                for nh in range(2):
                    nc.vector.tensor_copy(
                        out=xgT[0:16, nh * 512:(nh + 1) * 512],
                        in_=pxg[nh][0:16, :],
                    )
                pre_q = []
                for mi in range(MIQ):
                    pre = scr2.tile([128, 1027], mybir.dt.float32r,
                                    name="pre", tag="pre", bufs=4)
                    nc.gpsimd.memset(pre[:, 0:3].bitcast(F32), 0.0)
                    for nh in range(2):
                        if nh == 0:
                            nc.vector.tensor_copy(
                                out=pre[:, 3:3 + 512], in_=pq[mi * 2][:]
                            )
                        else:
                            nc.scalar.copy(
                                out=pre[:, 3 + 512:3 + 1024],
                                in_=pq[mi * 2 + 1][:],
                            )
                    pre_q.append(pre)
                # spT = softplus(-(xg @ Wg2 + bg2)) = ln(1 + exp(-logit))
                enxs = []
                for mi in range(MIQ):
                    for nh in range(2):
                        p = ps_proj.tile([128, 512], F32, name="pp_sp", tag="pp")
                        nc.tensor.matmul(
                            p[:],
                            wg2b_sb[:, mi * 128:(mi + 1) * 128],
                            xgT[:, nh * 512:(nh + 1) * 512],
                            start=True,
                            stop=True,
                        )
                        enx = scr2.tile([128, 512], F32, name="enx", tag="enx",
                                        bufs=4)
                        nc.scalar.activation(enx[:], p[:], AF.Exp, scale=-1.0)
                        enxs.append((mi, nh, enx))
                for mi, nh, enx in enxs:
                    nc.scalar.activation(
                        spT[:, mi, nh * 512:(nh + 1) * 512], enx[:],
                        AF.Ln, bias=1.0,
                    )
                # per-chunk inclusive cumsum of spT + chunk-end decay factors
                for mi in range(MIQ):
                    for c in range(NCH):
                        csl = slice(c * 128, (c + 1) * 128)
                        nc.vector.tensor_tensor_scan(
                            out=bsum[:, mi, csl],
                            data0=ones_sb[:],
                            data1=spT[:, mi, csl],
                            initial=0.0,
                            op0=OP.mult,
                            op1=OP.add,
                        )
                        nc.vector.tensor_scalar_mul(
                            bCn[:, mi, c:c + 1],
                            bsum[:, mi, c * 128 + 127:c * 128 + 128],
                            -1.0 / GATE_NORM,
                        )
                    nc.scalar.activation(Eall[:, mi, :], bCn[:, mi, :], AF.Exp)
                    # q-scale = exp(-b/16)/8 and k-scale = exp(b/16), full-T
                    nc.scalar.activation(
                        texp_all[:, mi, :], bsum[:, mi, :], AF.Exp,
                        scale=-1.0 / GATE_NORM, bias=negln8[:],
                    )
                    nc.scalar.activation(
                        texp_all[:, 2 + mi, :], bsum[:, mi, :], AF.Exp,
                        scale=1.0 / GATE_NORM,
                    )

                def conv_proj(w_sb, diag_base, dst, mi_count):
                    """dst[:, mi, :] = silu(conv4(src @ W[:, mi-block]))^T."""
                    for mi in range(mi_count):
                        pre = scr2.tile([128, 1027], mybir.dt.float32r,
                                        name="pre", tag="pre", bufs=4)
                        nc.gpsimd.memset(pre[:, 0:3].bitcast(F32), 0.0)
                        for nh in range(2):
                            p = ps_proj.tile([128, 512], F32, name="pp", tag="pp")
                            for kt in range(8):
                                nc.tensor.matmul(
                                    p[:],
                                    w_sb[:, kt, mi * 128:(mi + 1) * 128],
                                    srcT[:, kt, nh * 512:(nh + 1) * 512],
                                    start=(kt == 0),
                                    stop=(kt == 7),
                                )
                            nc.scalar.copy(
                                out=pre[:, 3 + nh * 512:3 + (nh + 1) * 512],
                                in_=p[:],
                            )
                        # causal conv: 4 shifted diag matmuls, then silu
                        for nh in range(2):
                            cp = ps_proj.tile([128, 512], F32, name="cp", tag="pp")
                            for j in range(CONV):
                                nc.tensor.matmul(
                                    cp[:],
                                    dg_all[:, (diag_base + mi) * CONV + j, :],
                                    pre[:, nh * 512 + j:nh * 512 + j + 512],
                                    start=(j == 0),
                                    stop=(j == 3),
                                )
                            sg = scr2.tile([128, 512], BF16, name="sg", tag="sg", bufs=4)
                            nc.scalar.activation(sg[:], cp[:], AF.Sigmoid)
                            nc.vector.tensor_mul(
                                dst[:, mi, nh * 512:(nh + 1) * 512], cp[:], sg[:]
                            )

                # q conv from the wave-1 pre tiles
                for mi in range(MIQ):
                    for nh in range(2):
                        cp = ps_proj.tile([128, 512], F32, name="cp", tag="pp")
                        for j in range(CONV):
                            nc.tensor.matmul(
                                cp[:],
                                dg_all[:, mi * CONV + j, :],
                                pre_q[mi][:, nh * 512 + j:nh * 512 + j + 512],
                                start=(j == 0),
                                stop=(j == 3),
                            )
                        sg = scr2.tile([128, 512], BF16, name="sg", tag="sg", bufs=4)
                        nc.scalar.activation(sg[:], cp[:], AF.Sigmoid)
                        nc.vector.tensor_mul(
                            q_sb[:, mi, nh * 512:(nh + 1) * 512], cp[:], sg[:]
                        )
                conv_proj(wk_sb, MIQ, k_sb, MIQ)

                # q~ = q * exp(-b/16)/8 and k~ = k * exp(b/16), in place
                for mi in range(MIQ):
                    for half in range(2):
                        hsl = slice(half * 512, (half + 1) * 512)
                        nc.vector.tensor_mul(
                            q_sb[:, mi, hsl], q_sb[:, mi, hsl],
                            texp_all[:, mi, hsl],
                        )
                        nc.vector.tensor_mul(
                            k_sb[:, mi, hsl], k_sb[:, mi, hsl],
                            texp_all[:, 2 + mi, hsl],
                        )

                conv_proj(wv_sb, 2 * MIQ, v_sb, MIV)

                # hoisted transposes: k^ and v into time-major layout
                for c in range(NCH):
                    csl = slice(c * 128, (c + 1) * 128)
                    for g in range(G):
                        kh_s = scr.tile([128, 128], BF16, name="kh_s", tag="kh_s")
                        nc.vector.tensor_scalar_mul(
                            kh_s[:], k_sb[:, g, csl], Eall[:, g, c:c + 1]
                        )
                        ps_k = ps_tr.tile([128, 128], BF16, name="ps_k", tag="pst")
                        nc.tensor.transpose(ps_k[:], kh_s[:], identb[:])
                        nc.scalar.copy(
                            out=khnat[:, c, g * 128:(g + 1) * 128], in_=ps_k[:]
                        )
                        ps_v = ps_tr.tile([128, 256], BF16, name="ps_v", tag="pst")
                        nc.tensor.matmul(
                            ps_v[:, 0:128], v_sb[:, 2 * g, csl], identb[:],
                            is_transpose=True, start=True, stop=False,
                            skip_group_check=True,
                        )
                        nc.tensor.matmul(
                            ps_v[:, 128:256], v_sb[:, 2 * g + 1, csl], identb[:],
                            is_transpose=True, start=False, stop=True,
                            skip_group_check=True,
                        )
                        nc.scalar.activation(
                            vnat[:, c, g * 256:(g + 1) * 256], ps_v[:],
                            AF.Copy, scale=maskc_sb[:, c:c + 1],
                        )

                # gate: silu(src @ Wgate), t-major (tile mt == chunk c)
                for mt in range(8):
                    p = ps_proj.tile([128, 512], F32, name="pp_gate", tag="pp")
                    for kt in range(8):
                        nc.tensor.matmul(
                            p[:],
                            srcT[:, kt, mt * 128:(mt + 1) * 128],
                            wgate_sb[:, kt, :],
                            start=(kt == 0),
                            stop=(kt == 7),
                        )
                    sgg = scr2.tile([128, 512], BF16, name="sgg", tag="sg", bufs=4)
                    nc.scalar.activation(sgg[:], p[:], AF.Sigmoid)
                    nc.vector.tensor_mul(gate_sb[:, mt, :], p[:], sgg[:])

            # ---- chunk recurrence + software-pipelined output tail ---------
            with (
                tc.tile_pool(name="ps_h", bufs=4, space="PSUM") as ps_h,
                tc.tile_pool(name="ps_o", bufs=2, space="PSUM") as ps_o_pool,
                tc.tile_pool(name="ps_out", bufs=2, space="PSUM") as ps_out,
                tc.tile_pool(name="stage", bufs=2) as stage_pool,
            ):

                def emit_gla(c):
                    csl = slice(c * 128, (c + 1) * 128)
                    for g in range(G):
                        # A~[s,t] per head via block-diagonal q operand
                        nc.vector.tensor_copy(
                            out=qblk[g][0:64, 0:128], in_=q_sb[0:64, g, csl]
                        )
                        nc.vector.tensor_copy(
                            out=qblk[g][64:128, 128:256],
                            in_=q_sb[64:128, g, csl],
                        )
                        ps_a = ps_h.tile([128, 256], F32, name="ps_a", tag="ps_h")
                        nc.tensor.matmul(
                            ps_a[:], k_sb[:, g, csl], qblk[g][:],
                            start=True, stop=True,
                        )
                        a_sb = scr.tile([128, 256], BF16, name="a_sb", tag="a_sb")
                        nc.vector.tensor_mul(a_sb[:], ps_a[:], triu2b[:])
                        # o = A~^T v (intra) + q~ @ S (inter)
                        ps_o = ps_o_pool.tile([128, 256], F32, name="ps_o",
                                              tag="ps_o")
                        nc.tensor.matmul(
                            ps_o[:, 0:128], a_sb[:, 0:128],
                            vnat[:, c, g * 256:g * 256 + 128],
                            start=True, stop=False, skip_group_check=True,
                        )
                        nc.tensor.matmul(
                            ps_o[:, 128:256], a_sb[:, 128:256],
                            vnat[:, c, g * 256 + 128:g * 256 + 256],
                            start=False, stop=False, skip_group_check=True,
                        )
                        nc.tensor.matmul(
                            ps_o[:], q_sb[:, g, csl], Sblk[g][:],
                            start=False, stop=True, skip_group_check=True,
                        )
                        # state update: S = diag(exp(b_C)) S + k^T v
                        ps_s = ps_h.tile([128, 256], F32, name="ps_s", tag="ps_h")
                        nc.tensor.matmul(
                            ps_s[:], khnat[:, c, g * 128:(g + 1) * 128],
                            vnat[:, c, g * 256:(g + 1) * 256],
                            start=True, stop=True,
                        )
                        nc.vector.scalar_tensor_tensor(
                            out=Sblk[g][0:64, 0:128],
                            in0=Sblk[g][0:64, 0:128],
                            scalar=Eall[0:64, g, c:c + 1],
                            in1=ps_s[0:64, 0:128],
                            op0=OP.mult,
                            op1=OP.add,
                        )
                        nc.vector.scalar_tensor_tensor(
                            out=Sblk[g][64:128, 128:256],
                            in0=Sblk[g][64:128, 128:256],
                            scalar=Eall[64:128, g, c:c + 1],
                            in1=ps_s[64:128, 128:256],
                            op0=OP.mult,
                            op1=OP.add,
                        )
                        # per-head sums of squares (pre-gate o), then fold the
                        # swish gate into gate_sb in place
                        for lh in range(2):
                            sq = scr.tile([128, 128], BF16, name="sq", tag="sq")
                            idx = c * 4 + 2 * g + lh
                            nc.scalar.activation(
                                sq[:], ps_o[:, lh * 128:(lh + 1) * 128],
                                AF.Square,
                                accum_out=ssq_all[:, idx:idx + 1],
                            )
                        gsl = slice(g * 256, (g + 1) * 256)
                        nc.vector.tensor_mul(
                            gate_sb[:, c, gsl], ps_o[:], gate_sb[:, c, gsl]
                        )

                def emit_tail(c):
                    csl = slice(c * 128, (c + 1) * 128)
                    # rrms = 1/sqrt(mean + eps); Sqrt/Square/Copy share one
                    # ACT table set so the loop never reloads tables
                    srt = scr.tile([128, 4], F32, name="srt", tag="lnr")
                    nc.scalar.activation(
                        srt[:], ssq_all[:, c * 4:(c + 1) * 4], AF.Sqrt,
                        scale=1.0 / DV, bias=eps_col[:],
                    )
                    nc.vector.reciprocal(
                        rrms_all[:, c * 4:(c + 1) * 4], srt[:]
                    )
                    for h in range(4):
                        nc.vector.tensor_scalar_mul(
                            gate_sb[:, c, h * 128:(h + 1) * 128],
                            gate_sb[:, c, h * 128:(h + 1) * 128],
                            rrms_all[:, c * 4 + h:c * 4 + h + 1],
                        )
                    for h in range(0, 4, 2):
                        ps_g = ps_h.tile([128, 256], BF16, name="ps_g",
                                         tag="ps_h")
                        nc.tensor.matmul(
                            ps_g[:, 0:128],
                            gate_sb[:, c, h * 128:(h + 1) * 128],
                            identb[:], is_transpose=True, start=True,
                            stop=False, skip_group_check=True,
                        )
                        nc.tensor.matmul(
                            ps_g[:, 128:256],
                            gate_sb[:, c, (h + 1) * 128:(h + 2) * 128],
                            identb[:], is_transpose=True, start=False,
                            stop=True, skip_group_check=True,
                        )
                        nc.scalar.copy(
                            out=ogT[:, h:h + 2, csl],
                            in_=ps_g[:].rearrange("p (a b) -> p a b", a=2),
                        )
                    stage = stage_pool.tile([128, D], BF16, name="stage",
                                            tag="stage")
                    for nh in range(2):
                        p = ps_out.tile([128, 512], F32, name="p_out",
                                        tag="p_out")
                        for h in range(4):
                            nc.tensor.matmul(
                                p[:],
                                ogT[:, h, csl],
                                wo_sb[:, h, nh * 512:(nh + 1) * 512],
                                start=(h == 0),
                                stop=(h == 3),
                            )
                        nc.scalar.copy(
                            out=stage[:, nh * 512:(nh + 1) * 512], in_=p[:]
                        )
                    nc.gpsimd.dma_start(out=out_d[c], in_=stage[:])

                for c in range(NCH):
                    emit_gla(c)
                    if c > 0:
                        emit_tail(c - 1)
                emit_tail(NCH - 1)

    nc.compile()
    return nc


_NC_CACHE = None


def _get_program():
    global _NC_CACHE
    if _NC_CACHE is None:
        _NC_CACHE = build_program()
    return _NC_CACHE


def _arr(x, nblk):
    """[nblk*128, m] f32 -> [128, nblk, m] bf16 (partition-major)."""
    m = x.shape[1]
    return np.ascontiguousarray(
        x.reshape(nblk, 128, m).transpose(1, 0, 2)
    ).astype(BF)


def shard_inputs(
    src, valid_mask, Wq, Wk, Wv, conv_q_w, conv_k_w, conv_v_w,
    Wg1, Wg2, bg2, Wgate, rms_w, Wo,
):
    """Build the 8 per-core input maps (bf16, SBUF-layout pre-arranged)."""
    f = np.float32
    src = np.asarray(src, f)
    valid_mask = np.asarray(valid_mask)
    in_maps = []
    wo_scaled = np.asarray(Wo, f) * np.tile(np.asarray(rms_w, f), VD // DV)[:, None]
    for core in range(NCORES):
        b, hg = core // 2, core % 2
        qs = slice(hg * KDC, (hg + 1) * KDC)
        vs = slice(hg * VDC, (hg + 1) * VDC)
        wg2b = np.concatenate(
            [np.asarray(Wg2, f)[:, qs], np.asarray(bg2, f)[None, qs]], axis=0
        )

        # conv tap table: [128, tile, 4] with tiles q(2), k(2), v(4)
        convw = np.zeros((128, 2 * MIQ + MIV, CONV), f)
        ti = 0
        for w, sel, n in ((conv_q_w, qs, MIQ), (conv_k_w, qs, MIQ),
                          (conv_v_w, vs, MIV)):
            wa = np.asarray(w, f)[sel]
            for i in range(n):
                convw[:, ti, :] = wa[i * 128:(i + 1) * 128]
                ti += 1

        in_maps.append({
            "srcT_in": _arr(np.ascontiguousarray(src[b].T), 8),
            "wq": _arr(np.asarray(Wq, f)[:, qs], 8),
            "wk": _arr(np.asarray(Wk, f)[:, qs], 8),
            "wv": _arr(np.asarray(Wv, f)[:, vs], 8),
            "wgate": _arr(np.asarray(Wgate, f)[:, vs], 8),
            "wg1": _arr(np.asarray(Wg1, f), 8),
            "wg2b": np.ascontiguousarray(wg2b).astype(BF),
            "wo": _arr(wo_scaled[vs, :], MIV),
            "convw": convw,
            "maskc": np.ascontiguousarray(
                valid_mask[b].astype(f).reshape(NCH, 128).T
            ),
        })
    return in_maps


def kernel(**inputs):
    nc = _get_program()
    in_maps = shard_inputs(**inputs)
    res = run_bass_kernel_spmd(nc, in_maps, list(range(NCORES)))
    out = np.zeros((B, T, D), np.float32)
    for core in range(NCORES):
        out[core // 2] += np.asarray(res.results[core]["out"],
                                     np.float32).reshape(T, D)
    return out


if __name__ == "__main__":
    prog = _get_program()
    print("program built OK")


# revision 52
# speedup vs baseline: 1.0275x; 1.0275x over previous
"""Gated Linear Attention forward on 8 Trainium2 NeuronCores (Bass/Tile).

Problem: B=4, T=1024, D=1024, H=8, DK=64, DV=128, conv4 on q/k/v, low-rank
log-sigmoid forget gate, recurrent scan, RMS-norm + swish output gate, out proj.

Sharding: core = 2*b + hg  (b = batch, hg = half of the heads).
Each core computes its batch's tokens for 4 heads end-to-end and a partial
output projection (Wo row-block); the host sums the two partials per batch.

v2 (bf16 rewrite of the f32r baseline):
- All matmul operands and most element-wise traffic are bf16 (PSUM stays f32),
  which removes the f32r small-free-dim 4x matmul penalty, halves LDWEIGHTS,
  doubles DVE throughput, and halves all DMA bytes.
- Every DRAM tensor is pre-arranged on the host into its exact SBUF layout so
  each DMA is 128 long contiguous descriptors (the baseline's 4KB-descriptor
  rearranges left the PE idle for the first 37us).
- silu runs as a single ACT Silu op straight out of PSUM; ACT ops are grouped
  by table set (natural_log_exp | silu | natural_log_exp) so the activation
  tables load ~3x instead of 15x.
- All v / k^ transposes are hoisted out of the recurrent chunk loop; the output
  projection for chunk c is emitted right after chunk c's recurrence so the PE
  stream stays dense (HAM stays un-throttled).
- conv runs on the PE as 4 shifted diagonal matmuls; the diagonal matrices are
  built on-device from a [128,8,4] tap table instead of DMAing 2MB of zeros.
"""

import numpy as np
import ml_dtypes

import concourse.bass as bass
import concourse.mybir as mybir
import concourse.tile as tile
from concourse import bacc
from concourse.bass_utils import run_bass_kernel_spmd

F32 = mybir.dt.float32
BF16 = mybir.dt.bfloat16
AF = mybir.ActivationFunctionType
OP = mybir.AluOpType

# problem constants (hardcoded per the task contract)
B, T, D, H = 4, 1024, 1024, 8
KD, VD = 512, 1024
DK, DV = 64, 128
CONV = 4
GATE_NORM = 16.0
EPS = 1e-5
LN8 = float(np.log(8.0))

# per-core shapes
KDC, VDC = 256, 512          # q/k and v/gate channels per core
MIQ, MIV = 2, 4              # 128-wide channel tiles for q/k and v
C, NCH = 128, 8              # chunk length, number of chunks
G = 2                        # head groups of 2 heads (128 chans) per core
NCORES = 8

BF = ml_dtypes.bfloat16


def build_program():
    nc = bacc.Bacc("TRN2", target_bir_lowering=False, debug=False)

    # ---- DRAM I/O (all host-pre-arranged to SBUF layout, bf16) -------------
    srcT_d = nc.dram_tensor("srcT_in", [128, 8, T], BF16, kind="ExternalInput")
    wq_d = nc.dram_tensor("wq", [128, 8, KDC], BF16, kind="ExternalInput")
    wk_d = nc.dram_tensor("wk", [128, 8, KDC], BF16, kind="ExternalInput")
    wv_d = nc.dram_tensor("wv", [128, 8, VDC], BF16, kind="ExternalInput")
    wgate_d = nc.dram_tensor("wgate", [128, 8, VDC], BF16, kind="ExternalInput")
    wg1_d = nc.dram_tensor("wg1", [128, 8, 16], BF16, kind="ExternalInput")
    wg2b_d = nc.dram_tensor("wg2b", [17, KDC], BF16, kind="ExternalInput")
    wo_d = nc.dram_tensor("wo", [128, MIV, D], BF16, kind="ExternalInput")
    convw_d = nc.dram_tensor("convw", [128, 2 * MIQ + MIV, CONV], F32,
                             kind="ExternalInput")
    maskc_d = nc.dram_tensor("maskc", [128, NCH], F32, kind="ExternalInput")
    out_d = nc.dram_tensor("out", [NCH, 128, D], BF16, kind="ExternalOutput")

    ident_np = np.eye(128, dtype=np.float32)
    u = np.triu(np.ones((128, 128), np.float32))  # U[s,t] = 1 iff s <= t
    ident_d = nc.inline_tensor(ident_np, "ident_c")
    triu2_d = nc.inline_tensor(np.concatenate([u, u], axis=1), "triu2_c")

    # ---- static SBUF -------------------------------------------------------
    srcT = nc.alloc_sbuf_tensor("srcT", [128, 8, T], BF16)       # src^T, d-major
    q_sb = nc.alloc_sbuf_tensor("q_sb", [128, MIQ, T], BF16)     # q then q~
    k_sb = nc.alloc_sbuf_tensor("k_sb", [128, MIQ, T], BF16)     # k then k~
    v_sb = nc.alloc_sbuf_tensor("v_sb", [128, MIV, T], BF16)     # chan-major v
    gate_sb = nc.alloc_sbuf_tensor("gate_sb", [128, NCH, VDC], BF16)
    vnat = nc.alloc_sbuf_tensor("vnat", [128, NCH, VDC], BF16)   # time-major v
    khnat = nc.alloc_sbuf_tensor("khnat", [128, NCH, KDC], BF16)  # time-major k^
    xgT = nc.alloc_sbuf_tensor("xgT", [17, T], BF16)             # (src@Wg1)^T+1s
    spT = nc.alloc_sbuf_tensor("spT", [128, MIQ, T], F32)        # softplus(-gk)
    bsum = nc.alloc_sbuf_tensor("bsum", [128, MIQ, T], F32)      # chunk cumsum
    bCn = nc.alloc_sbuf_tensor("bCn", [128, MIQ, NCH], F32)
    Eall = nc.alloc_sbuf_tensor("Eall", [128, MIQ, NCH], F32)    # exp(b_C)
    texp_all = nc.alloc_sbuf_tensor("texp_all", [128, 4, T], BF16)
    ssq_all = nc.alloc_sbuf_tensor("ssq_all", [128, NCH * 4], F32)
    rrms_all = nc.alloc_sbuf_tensor("rrms_all", [128, NCH * 4], F32)
    wo_sb = nc.alloc_sbuf_tensor("wo_sb", [128, MIV, D], BF16)
    wgate_sb = nc.alloc_sbuf_tensor("wgate_sb", [128, 8, VDC], BF16)
    wg1_sb = nc.alloc_sbuf_tensor("wg1_sb", [128, 8, 16], BF16)
    wg2b_sb = nc.alloc_sbuf_tensor("wg2b_sb", [17, KDC], BF16)
    convw_sb = nc.alloc_sbuf_tensor("convw_sb", [128, 2 * MIQ + MIV, CONV], F32)
    maskc_sb = nc.alloc_sbuf_tensor("maskc_sb", [128, NCH], F32)
    ident_f = nc.alloc_sbuf_tensor("ident_f", [128, 128], F32)
    triu2_f = nc.alloc_sbuf_tensor("triu2_f", [128, 256], F32)
    identb = nc.alloc_sbuf_tensor("identb", [128, 128], BF16)
    triu2b = nc.alloc_sbuf_tensor("triu2b", [128, 256], BF16)
    dg_all = nc.alloc_sbuf_tensor("dg_all", [128, (2 * MIQ + MIV) * CONV, 128],
                                  mybir.dt.float32r)
    ones_sb = nc.alloc_sbuf_tensor("ones_sb", [128, 128], F32)
    ogT = nc.alloc_sbuf_tensor("ogT", [128, MIV, T], BF16)
    Sblk = [nc.alloc_sbuf_tensor(f"Sblk{g}", [128, 256], BF16) for g in range(G)]
    qblk = [nc.alloc_sbuf_tensor(f"qblk{g}", [128, 256], BF16) for g in range(G)]
    negln8 = nc.alloc_sbuf_tensor("negln8", [128, 1], F32)
    eps_col = nc.alloc_sbuf_tensor("eps_col", [128, 1], F32)

    with tile.TileContext(nc) as tc:
        with (
            tc.tile_pool(name="scr", bufs=4) as scr,
        ):
            # ---- phase 0: constants + input DMAs (one straight transfer per
            # tensor; src split by partition quarters across four queues) ----
            nc.sync.dma_start(out=ident_f[:], in_=ident_d[:])
            nc.sync.dma_start(out=triu2_f[:], in_=triu2_d[:])
            nc.sync.dma_start(out=wg1_sb[:], in_=wg1_d[:])
            nc.scalar.dma_start(out=wg2b_sb[:], in_=wg2b_d[:])
            # src sliced by kt pairs so per-kt accumulation starts early
            nc.sync.dma_start(out=srcT[:, 0:2, :], in_=srcT_d[:, 0:2, :])
            nc.scalar.dma_start(out=srcT[:, 2:4, :], in_=srcT_d[:, 2:4, :])
            nc.gpsimd.dma_start(out=srcT[:, 4:6, :], in_=srcT_d[:, 4:6, :])
            nc.sync.dma_start(out=srcT[:, 6:8, :], in_=srcT_d[:, 6:8, :])
            nc.scalar.dma_start(out=convw_sb[:], in_=convw_d[:])
            nc.sync.dma_start(out=maskc_sb[:], in_=maskc_d[:])

            nc.vector.memset(ones_sb[:], 1.0)
            nc.vector.memset(negln8[:], -LN8)
            nc.vector.memset(eps_col[:], EPS)
            # row 16 is the bias ones-row; rows 0..15 are overwritten later
            nc.vector.memset(xgT[:], 1.0)
            for g in range(G):
                nc.vector.memset(Sblk[g][:], 0.0)
                nc.vector.memset(qblk[g][:], 0.0)
            nc.vector.tensor_copy(out=identb[:], in_=ident_f[:])
            nc.vector.tensor_copy(out=triu2b[:], in_=triu2_f[:])

            wq_sb = nc.alloc_sbuf_tensor("wq_sb", [128, 8, KDC], BF16)
            wk_sb = nc.alloc_sbuf_tensor("wk_sb", [128, 8, KDC], BF16)
            wv_sb = nc.alloc_sbuf_tensor("wv_sb", [128, 8, VDC], BF16)
            nc.sync.dma_start(out=wq_sb[:], in_=wq_d[:])
            nc.scalar.dma_start(out=wk_sb[:], in_=wk_d[:])
            nc.gpsimd.dma_start(out=wv_sb[:], in_=wv_d[:])
            nc.sync.dma_start(out=wgate_sb[:], in_=wgate_d[:])
            nc.gpsimd.dma_start(out=wo_sb[:], in_=wo_d[:])

            # conv diag matrices built on-device: dg[ti*4+j] = diag(w[:, ti, j])
            for ti in range(2 * MIQ + MIV):
                for j in range(CONV):
                    nc.vector.tensor_scalar_mul(
                        dg_all[:, ti * CONV + j, :], ident_f[:],
                        convw_sb[:, ti, j:j + 1],
                    )

            # ---- gk path + projections + conv + gate -----------------------
            with (
                tc.tile_pool(name="scr2", bufs=2) as scr2,
                tc.tile_pool(name="ps_proj", bufs=6, space="PSUM") as ps_proj,
                tc.tile_pool(name="ps_tr", bufs=2, space="PSUM") as ps_tr,
            ):
                # kt-wave: xg and the q projection accumulate together so
                # each arriving src kt-slice feeds 6 back-to-back matmuls
                pxg = [ps_proj.tile([128, 512], F32, name="pp_xg", tag="pp")
                       for _ in range(2)]
                pq = [ps_proj.tile([128, 512], F32, name="pp_q", tag="pp")
                      for _ in range(4)]
                for kt in range(8):
                    for nh in range(2):
                        nc.tensor.matmul(
                            pxg[nh][0:16, :],
                            wg1_sb[:, kt, :],
                            srcT[:, kt, nh * 512:(nh + 1) * 512],
                            start=(kt == 0),
                            stop=(kt == 7),
                        )
                    for mi in range(MIQ):
                        for nh in range(2):
                            nc.tensor.matmul(
                                pq[mi * 2 + nh][:],
                                wq_sb[:, kt, mi * 128:(mi + 1) * 128],
                                srcT[:, kt, nh * 512:(nh + 1) * 512],
                                start=(kt == 0),
                                stop=(kt == 7),
                            )
                for nh in range(2):
                    nc.vector.tensor_copy(
                        out=xgT[0:16, nh * 512:(nh + 1) * 512],
                        in_=pxg[nh][0:16, :],
                    )
                pre_q = []
                for mi in range(MIQ):
                    pre = scr2.tile([128, 1027], mybir.dt.float32r,
                                    name="pre", tag="pre")
                    nc.gpsimd.memset(pre[:, 0:3].bitcast(F32), 0.0)
                    for nh in range(2):
                        if nh == 0:
                            nc.vector.tensor_copy(
                                out=pre[:, 3:3 + 512], in_=pq[mi * 2][:]
                            )
                        else:
                            nc.scalar.copy(
                                out=pre[:, 3 + 512:3 + 1024],
                                in_=pq[mi * 2 + 1][:],
                            )
                    pre_q.append(pre)
                # spT = softplus(-(xg @ Wg2 + bg2)) = ln(1 + exp(-logit))
                enxs = []
                for mi in range(MIQ):
                    for nh in range(2):
                        p = ps_proj.tile([128, 512], F32, name="pp_sp", tag="pp")
                        nc.tensor.matmul(
                            p[:],
                            wg2b_sb[:, mi * 128:(mi + 1) * 128],
                            xgT[:, nh * 512:(nh + 1) * 512],
                            start=True,
                            stop=True,
                        )
                        enx = scr2.tile([128, 512], F32, name="enx", tag="enx",
                                        bufs=4)
                        nc.scalar.activation(enx[:], p[:], AF.Exp, scale=-1.0)
                        enxs.append((mi, nh, enx))
                for mi, nh, enx in enxs:
                    nc.scalar.activation(
                        spT[:, mi, nh * 512:(nh + 1) * 512], enx[:],
                        AF.Ln, bias=1.0,
                    )
                # per-chunk inclusive cumsum of spT + chunk-end decay factors
                for mi in range(MIQ):
                    for c in range(NCH):
                        csl = slice(c * 128, (c + 1) * 128)
                        nc.vector.tensor_tensor_scan(
                            out=bsum[:, mi, csl],
                            data0=ones_sb[:],
                            data1=spT[:, mi, csl],
                            initial=0.0,
                            op0=OP.mult,
                            op1=OP.add,
                        )
                        nc.vector.tensor_scalar_mul(
                            bCn[:, mi, c:c + 1],
                            bsum[:, mi, c * 128 + 127:c * 128 + 128],
                            -1.0 / GATE_NORM,
                        )
                    nc.scalar.activation(Eall[:, mi, :], bCn[:, mi, :], AF.Exp)
                    # q-scale = exp(-b/16)/8 and k-scale = exp(b/16), full-T
                    nc.scalar.activation(
                        texp_all[:, mi, :], bsum[:, mi, :], AF.Exp,
                        scale=-1.0 / GATE_NORM, bias=negln8[:],
                    )
                    nc.scalar.activation(
                        texp_all[:, 2 + mi, :], bsum[:, mi, :], AF.Exp,
                        scale=1.0 / GATE_NORM,
                    )

                def conv_proj(w_sb, diag_base, dst, mi_count):
                    """dst[:, mi, :] = silu(conv4(src @ W[:, mi-block]))^T."""
                    for mi in range(mi_count):
                        pre = scr2.tile([128, 1027], mybir.dt.float32r,
                                        name="pre", tag="pre")
                        nc.gpsimd.memset(pre[:, 0:3].bitcast(F32), 0.0)
                        for nh in range(2):
                            p = ps_proj.tile([128, 512], F32, name="pp", tag="pp")
                            for kt in range(8):
                                nc.tensor.matmul(
                                    p[:],
                                    w_sb[:, kt, mi * 128:(mi + 1) * 128],
                                    srcT[:, kt, nh * 512:(nh + 1) * 512],
                                    start=(kt == 0),
                                    stop=(kt == 7),
                                )
                            nc.scalar.copy(
                                out=pre[:, 3 + nh * 512:3 + (nh + 1) * 512],
                                in_=p[:],
                            )
                        # causal conv: 4 shifted diag matmuls, then silu
                        for nh in range(2):
                            cp = ps_proj.tile([128, 512], F32, name="cp", tag="pp")
                            for j in range(CONV):
                                nc.tensor.matmul(
                                    cp[:],
                                    dg_all[:, (diag_base + mi) * CONV + j, :],
                                    pre[:, nh * 512 + j:nh * 512 + j + 512],
                                    start=(j == 0),
                                    stop=(j == 3),
                                )
                            sg = scr2.tile([128, 512], BF16, name="sg", tag="sg")
                            nc.scalar.activation(sg[:], cp[:], AF.Sigmoid)
                            nc.vector.tensor_mul(
                                dst[:, mi, nh * 512:(nh + 1) * 512], cp[:], sg[:]
                            )

                # q conv from the wave-1 pre tiles
                for mi in range(MIQ):
                    for nh in range(2):
                        cp = ps_proj.tile([128, 512], F32, name="cp", tag="pp")
                        for j in range(CONV):
                            nc.tensor.matmul(
                                cp[:],
                                dg_all[:, mi * CONV + j, :],
                                pre_q[mi][:, nh * 512 + j:nh * 512 + j + 512],
                                start=(j == 0),
                                stop=(j == 3),
                            )
                        sg = scr2.tile([128, 512], BF16, name="sg", tag="sg")
                        nc.scalar.activation(sg[:], cp[:], AF.Sigmoid)
                        nc.vector.tensor_mul(
                            q_sb[:, mi, nh * 512:(nh + 1) * 512], cp[:], sg[:]
                        )
                conv_proj(wk_sb, MIQ, k_sb, MIQ)

                # q~ = q * exp(-b/16)/8 and k~ = k * exp(b/16), in place
                for mi in range(MIQ):
                    for half in range(2):
                        hsl = slice(half * 512, (half + 1) * 512)
                        nc.vector.tensor_mul(
                            q_sb[:, mi, hsl], q_sb[:, mi, hsl],
                            texp_all[:, mi, hsl],
                        )
                        nc.vector.tensor_mul(
                            k_sb[:, mi, hsl], k_sb[:, mi, hsl],
                            texp_all[:, 2 + mi, hsl],
                        )

                conv_proj(wv_sb, 2 * MIQ, v_sb, MIV)

                # hoisted transposes: k^ and v into time-major layout
                for c in range(NCH):
                    csl = slice(c * 128, (c + 1) * 128)
                    for g in range(G):
                        kh_s = scr.tile([128, 128], BF16, name="kh_s", tag="kh_s")
                        nc.vector.tensor_scalar_mul(
                            kh_s[:], k_sb[:, g, csl], Eall[:, g, c:c + 1]
                        )
                        ps_k = ps_tr.tile([128, 128], BF16, name="ps_k", tag="pst")
                        nc.tensor.transpose(ps_k[:], kh_s[:], identb[:])
                        nc.scalar.copy(
                            out=khnat[:, c, g * 128:(g + 1) * 128], in_=ps_k[:]
                        )
                        ps_v = ps_tr.tile([128, 256], BF16, name="ps_v", tag="pst")
                        nc.tensor.matmul(
                            ps_v[:, 0:128], v_sb[:, 2 * g, csl], identb[:],
                            is_transpose=True, start=True, stop=False,
                            skip_group_check=True,
                        )
                        nc.tensor.matmul(
                            ps_v[:, 128:256], v_sb[:, 2 * g + 1, csl], identb[:],
                            is_transpose=True, start=False, stop=True,
                            skip_group_check=True,
                        )
                        nc.scalar.activation(
                            vnat[:, c, g * 256:(g + 1) * 256], ps_v[:],
                            AF.Copy, scale=maskc_sb[:, c:c + 1],
                        )

                # gate: silu(src @ Wgate), t-major (tile mt == chunk c)
                for mt in range(8):
                    p = ps_proj.tile([128, 512], F32, name="pp_gate", tag="pp")
                    for kt in range(8):
                        nc.tensor.matmul(
                            p[:],
                            srcT[:, kt, mt * 128:(mt + 1) * 128],
                            wgate_sb[:, kt, :],
                            start=(kt == 0),
                            stop=(kt == 7),
                        )
                    sgg = scr2.tile([128, 512], BF16, name="sgg", tag="sg")
                    nc.scalar.activation(sgg[:], p[:], AF.Sigmoid)
                    nc.vector.tensor_mul(gate_sb[:, mt, :], p[:], sgg[:])

            # ---- chunk recurrence + software-pipelined output tail ---------
            with (
                tc.tile_pool(name="ps_h", bufs=4, space="PSUM") as ps_h,
                tc.tile_pool(name="ps_o", bufs=2, space="PSUM") as ps_o_pool,
                tc.tile_pool(name="ps_out", bufs=2, space="PSUM") as ps_out,
                tc.tile_pool(name="stage", bufs=2) as stage_pool,
            ):

                def emit_gla(c):
                    csl = slice(c * 128, (c + 1) * 128)
                    for g in range(G):
                        # A~[s,t] per head via block-diagonal q operand
                        nc.vector.tensor_copy(
                            out=qblk[g][0:64, 0:128], in_=q_sb[0:64, g, csl]
                        )
                        nc.vector.tensor_copy(
                            out=qblk[g][64:128, 128:256],
                            in_=q_sb[64:128, g, csl],
                        )
                        ps_a = ps_h.tile([128, 256], F32, name="ps_a", tag="ps_h")
                        nc.tensor.matmul(
                            ps_a[:], k_sb[:, g, csl], qblk[g][:],
                            start=True, stop=True,
                        )
                        a_sb = scr.tile([128, 256], BF16, name="a_sb", tag="a_sb")
                        nc.vector.tensor_mul(a_sb[:], ps_a[:], triu2b[:])
                        # o = A~^T v (intra) + q~ @ S (inter)
                        ps_o = ps_o_pool.tile([128, 256], F32, name="ps_o",
                                              tag="ps_o")
                        nc.tensor.matmul(
                            ps_o[:, 0:128], a_sb[:, 0:128],
                            vnat[:, c, g * 256:g * 256 + 128],
                            start=True, stop=False, skip_group_check=True,
                        )
                        nc.tensor.matmul(
                            ps_o[:, 128:256], a_sb[:, 128:256],
                            vnat[:, c, g * 256 + 128:g * 256 + 256],
                            start=False, stop=False, skip_group_check=True,
                        )
                        nc.tensor.matmul(
                            ps_o[:], q_sb[:, g, csl], Sblk[g][:],
                            start=False, stop=True, skip_group_check=True,
                        )
                        # state update: S = diag(exp(b_C)) S + k^T v
                        ps_s = ps_h.tile([128, 256], F32, name="ps_s", tag="ps_h")
                        nc.tensor.matmul(
                            ps_s[:], khnat[:, c, g * 128:(g + 1) * 128],
                            vnat[:, c, g * 256:(g + 1) * 256],
                            start=True, stop=True,
                        )
                        nc.vector.scalar_tensor_tensor(
                            out=Sblk[g][0:64, 0:128],
                            in0=Sblk[g][0:64, 0:128],
                            scalar=Eall[0:64, g, c:c + 1],
                            in1=ps_s[0:64, 0:128],
                            op0=OP.mult,
                            op1=OP.add,
                        )
                        nc.vector.scalar_tensor_tensor(
                            out=Sblk[g][64:128, 128:256],
                            in0=Sblk[g][64:128, 128:256],
                            scalar=Eall[64:128, g, c:c + 1],
                            in1=ps_s[64:128, 128:256],
                            op0=OP.mult,
                            op1=OP.add,
                        )
                        # per-head sums of squares (pre-gate o), then fold the
                        # swish gate into gate_sb in place
                        for lh in range(2):
                            sq = scr.tile([128, 128], BF16, name="sq", tag="sq")
                            idx = c * 4 + 2 * g + lh
                            nc.scalar.activation(
                                sq[:], ps_o[:, lh * 128:(lh + 1) * 128],
                                AF.Square,
                                accum_out=ssq_all[:, idx:idx + 1],
                            )
                        gsl = slice(g * 256, (g + 1) * 256)
                        nc.vector.tensor_mul(
                            gate_sb[:, c, gsl], ps_o[:], gate_sb[:, c, gsl]
                        )

                def emit_tail(c):
                    csl = slice(c * 128, (c + 1) * 128)
                    # rrms = 1/sqrt(mean + eps); Sqrt/Square/Copy share one
                    # ACT table set so the loop never reloads tables
                    srt = scr.tile([128, 4], F32, name="srt", tag="lnr")
                    nc.scalar.activation(
                        srt[:], ssq_all[:, c * 4:(c + 1) * 4], AF.Sqrt,
                        scale=1.0 / DV, bias=eps_col[:],
                    )
                    nc.vector.reciprocal(
                        rrms_all[:, c * 4:(c + 1) * 4], srt[:]
                    )
                    for h in range(4):
                        nc.vector.tensor_scalar_mul(
                            gate_sb[:, c, h * 128:(h + 1) * 128],
                            gate_sb[:, c, h * 128:(h + 1) * 128],
                            rrms_all[:, c * 4 + h:c * 4 + h + 1],
                        )
                    for h in range(0, 4, 2):
                        ps_g = ps_h.tile([128, 256], BF16, name="ps_g",
                                         tag="ps_h")
                        nc.tensor.matmul(
                            ps_g[:, 0:128],
                            gate_sb[:, c, h * 128:(h + 1) * 128],
                            identb[:], is_transpose=True, start=True,
                            stop=False, skip_group_check=True,
                        )
                        nc.tensor.matmul(
                            ps_g[:, 128:256],
                            gate_sb[:, c, (h + 1) * 128:(h + 2) * 128],
                            identb[:], is_transpose=True, start=False,
                            stop=True, skip_group_check=True,
                        )
                        nc.scalar.copy(
                            out=ogT[:, h:h + 2, csl],
                            in_=ps_g[:].rearrange("p (a b) -> p a b", a=2),
                        )
                    stage = stage_pool.tile([128, D], BF16, name="stage",
                                            tag="stage")
                    for nh in range(2):
                        p = ps_out.tile([128, 512], F32, name="p_out",
                                        tag="p_out")
                        for h in range(4):
                            nc.tensor.matmul(
                                p[:],
                                ogT[:, h, csl],
                                wo_sb[:, h, nh * 512:(nh + 1) * 512],
                                start=(h == 0),
                                stop=(h == 3),
                            )
                        nc.scalar.copy(
                            out=stage[:, nh * 512:(nh + 1) * 512], in_=p[:]
                        )
                    nc.gpsimd.dma_start(out=out_d[c], in_=stage[:])

                for c in range(NCH):
                    emit_gla(c)
                    if c > 0:
                        emit_tail(c - 1)
                emit_tail(NCH - 1)

    nc.compile()
    return nc


_NC_CACHE = None


def _get_program():
    global _NC_CACHE
    if _NC_CACHE is None:
        _NC_CACHE = build_program()
    return _NC_CACHE


def _arr(x, nblk):
    """[nblk*128, m] f32 -> [128, nblk, m] bf16 (partition-major)."""
    m = x.shape[1]
    return np.ascontiguousarray(
        x.reshape(nblk, 128, m).transpose(1, 0, 2)
    ).astype(BF)


def shard_inputs(
    src, valid_mask, Wq, Wk, Wv, conv_q_w, conv_k_w, conv_v_w,
    Wg1, Wg2, bg2, Wgate, rms_w, Wo,
):
    """Build the 8 per-core input maps (bf16, SBUF-layout pre-arranged)."""
    f = np.float32
    src = np.asarray(src, f)
    valid_mask = np.asarray(valid_mask)
    in_maps = []
    wo_scaled = np.asarray(Wo, f) * np.tile(np.asarray(rms_w, f), VD // DV)[:, None]
    for core in range(NCORES):
        b, hg = core // 2, core % 2
        qs = slice(hg * KDC, (hg + 1) * KDC)
        vs = slice(hg * VDC, (hg + 1) * VDC)
        wg2b = np.concatenate(
            [np.asarray(Wg2, f)[:, qs], np.asarray(bg2, f)[None, qs]], axis=0
        )

        # conv tap table: [128, tile, 4] with tiles q(2), k(2), v(4)
        convw = np.zeros((128, 2 * MIQ + MIV, CONV), f)
        ti = 0
        for w, sel, n in ((conv_q_w, qs, MIQ), (conv_k_w, qs, MIQ),
                          (conv_v_w, vs, MIV)):
            wa = np.asarray(w, f)[sel]
            for i in range(n):
                convw[:, ti, :] = wa[i * 128:(i + 1) * 128]
                ti += 1

        in_maps.append({
            "srcT_in": _arr(np.ascontiguousarray(src[b].T), 8),
            "wq": _arr(np.asarray(Wq, f)[:, qs], 8),
            "wk": _arr(np.asarray(Wk, f)[:, qs], 8),
            "wv": _arr(np.asarray(Wv, f)[:, vs], 8),
            "wgate": _arr(np.asarray(Wgate, f)[:, vs], 8),
            "wg1": _arr(np.asarray(Wg1, f), 8),
            "wg2b": np.ascontiguousarray(wg2b).astype(BF),
            "wo": _arr(wo_scaled[vs, :], MIV),
            "convw": convw,
            "maskc": np.ascontiguousarray(
                valid_mask[b].astype(f).reshape(NCH, 128).T
            ),
        })
    return in_maps


def kernel(**inputs):
    nc = _get_program()
    in_maps = shard_inputs(**inputs)
    res = run_bass_kernel_spmd(nc, in_maps, list(range(NCORES)))
    out = np.zeros((B, T, D), np.float32)
    for core in range(NCORES):
        out[core // 2] += np.asarray(res.results[core]["out"],
                                     np.float32).reshape(T, D)
    return out


if __name__ == "__main__":
    prog = _get_program()
    print("program built OK")
